# revision 1
# baseline (speedup 1.0000x reference)
"""Trainium2 Bass kernel: 4-head attention (nn_Attention_75960791598018).

Full inputs in, full outputs out. The batch dim (n=8) is sharded 1:1 across
the 8 NeuronCores (pure data parallelism, no collectives).

Per-core dataflow (x_i: [2048, 1024]):
  xT = transpose(x)                        (PE transpose, 128x128 tiles)
  QT[h] = W_Q[h].T @ xT   [dh, S]          (W stationary, xT moving)
  KT[h] = W_K[h].T @ xT   [dh, S]
  V[h]  = x @ W_V[h]      [S, dh]          (xT slices stationary, W moving)
  scoresT[h] = KT.T-tiles @ QT  [k, q]     (k on partitions)
  E = exp(scoresT / sqrt(dh))              (no max subtraction; scores ~ +-3)
  den[q] = sum_k E                         (DVE accumulate chains + GpSimd
                                            partition_all_reduce; off the PE)
  zTu[h] = V-tiles.T @ E   [dh, q]
  zT[h] = zTu * (1/den)                    (normalization commutes past P@V)
  outT[d, s] = W_O.T-tiles @ zT + cb[d]    (contract all heads; bias is
                                            per-partition in this layout)
  host: out = outT.T;  cb = b_O + b_V . W_O  (folded on host)

b_Q / b_K are applied as per-partition ACT biases on the QT/KT evacuations.
All matmuls run in float32r (tf32-class mantissa, 1 cycle/row at N>=256 vs 4
for fp32; measured per-matmul rel err 1.7e-4 vs 2.3e-3 for bf16).

Note: DMAs touching 1-partition SBUF rows ([1, N] tiles) fail to load /
crash the exec unit in this environment — everything here moves full
128-partition tiles.
"""

import os
from contextlib import ExitStack

import numpy as np

import concourse.bass as bass
import concourse.bass_isa as bass_isa
from concourse import bacc
import concourse.mybir as mybir
import concourse.tile as tile
from concourse.bass_utils import run_bass_kernel_spmd

S, D, H, DH = 2048, 1024, 4, 256
P = 128
NT_S = S // P          # 16 s-tiles
NT_D = D // P          # 8 d-tiles
NT_E = DH // P         # 2 e-tiles per head
QC = 512               # q-chunk width
NQC = S // QC          # 4
NHE = (H * DH) // P    # 8 (h,e) tiles
F32 = mybir.dt.float32
F32R = mybir.dt.float32r
SCALE = 1.0 / 16.0     # 1/sqrt(DH)
N_CORES = 8

Act = mybir.ActivationFunctionType


def _build():
    n_heads = int(os.environ.get("KBUILD_HEADS", str(H)))
    do_c = os.environ.get("KBUILD_PHASE_C", "1") == "1"
    n_ts = int(os.environ.get("KBUILD_NTS", str(NT_S)))
    reps = int(os.environ.get("KBENCH_REPS", "1"))

    nc = bacc.Bacc("TRN2", target_bir_lowering=False, debug=False)
    x = nc.dram_tensor("x", [S, D], F32R, kind="ExternalInput").ap()
    idin = nc.dram_tensor("idin", [P, P], F32, kind="ExternalInput").ap()
    wq = nc.dram_tensor("wq", [H, D, DH], F32R, kind="ExternalInput").ap()
    wk = nc.dram_tensor("wk", [H, D, DH], F32R, kind="ExternalInput").ap()
    wv = nc.dram_tensor("wv", [H, D, DH], F32R, kind="ExternalInput").ap()
    wo = nc.dram_tensor("wo", [H, DH, D], F32R, kind="ExternalInput").ap()
    bq = nc.dram_tensor("bq", [H, DH], F32, kind="ExternalInput").ap()
    bk = nc.dram_tensor("bk", [H, DH], F32, kind="ExternalInput").ap()
    cb = nc.dram_tensor("cb", [D], F32, kind="ExternalInput").ap()
    outT = nc.dram_tensor("outT", [D, S], F32, kind="ExternalOutput").ap()
    # normalized z^T scratch, [(h e), s]
    zn = nc.dram_tensor("zn", [H * DH, S], F32R).ap()

    wo_he = wo.rearrange("h e d -> (h e) d")

    with tile.TileContext(nc) as tc, ExitStack() as ctx:
        misc = ctx.enter_context(tc.tile_pool(name="misc", bufs=1))

        ident_f = misc.tile([P, P], F32)
        nc.sync.dma_start(out=ident_f, in_=idin)
        # f32r identity: exact values, and the transpose-mode cost is keyed
        # to the moving (identity) dtype -> 1.5 cycles/row instead of 2
        ident = misc.tile([P, P], F32R)
        nc.vector.tensor_copy(out=ident, in_=ident_f)

        bq_sb = misc.tile([P, H * NT_E], F32)     # [128, (h et)]
        nc.gpsimd.dma_start(out=bq_sb,
                            in_=bq.rearrange("h (t p) -> p (h t)", p=P))
        bk_sb = misc.tile([P, H * NT_E], F32)
        nc.gpsimd.dma_start(out=bk_sb,
                            in_=bk.rearrange("h (t p) -> p (h t)", p=P))
        cb_sb = misc.tile([P, NT_D], F32)         # [128, d-tile]
        nc.gpsimd.dma_start(out=cb_sb, in_=cb.rearrange("(t p) -> p t", p=P))
        # phase-C warm-start tiles: live in the never-released misc pool so
        # their DMAs aren't blocked on head-loop pool address reuse
        wo0_pre = misc.tile([P, D], F32R)

        xt_pool = ctx.enter_context(tc.tile_pool(name="xt", bufs=1))

        for rep in range(reps):
          xts = [xt_pool.tile([P, S], F32R, name=f"xt{d}_{rep}", tag=f"xt{d}")
                 for d in range(NT_D)]
          with (
              tc.tile_pool(name="xstage", bufs=3) as xstage,
              tc.tile_pool(name="ps1", bufs=1, space="PSUM") as ps1,
              tc.tile_pool(name="wp", bufs=1) as wp,
              tc.tile_pool(name="qkv", bufs=1) as qkv,
              tc.tile_pool(name="ep", bufs=1) as ep,
              tc.tile_pool(name="work", bufs=1) as work,
          ):
              # head-0 weight tiles, loaded interleaved with the x stream so
              # head-0 projections can start while x is still arriving
              w0 = {}
              for nm in ("wq", "wk", "wv"):
                  w0[nm] = [wp.tile([P, 4 * DH], F32R, name=f"{nm}0_{g}_{rep}",
                                    tag=f"{nm}{g}") for g in range(2)]

              def emit_w0(g):
                  for nm, wsrc in (("wq", wq), ("wk", wk), ("wv", wv)):
                      nc.sync.dma_start(
                          out=w0[nm][g].rearrange("p (t e) -> p t e", t=4),
                          in_=wsrc[0, g * 512:(g + 1) * 512, :].rearrange(
                              "(t p) e -> p t e", p=P))

              # ---- phase A: transpose x into xT (f32r) ----
              for st in range(n_ts):
                  xs = xstage.tile([P, D], F32R, name=f"xs{st}", tag="xs")
                  nc.sync.dma_start(out=xs, in_=x[st * P:(st + 1) * P, :])
                  if st == 6:
                      emit_w0(0)
                  if st == 10:
                      emit_w0(1)
                  for d in range(NT_D):
                      pt = ps1.tile([P, P], F32R, name=f"pt{st}_{d}", tag="z", bufs=3)
                      nc.tensor.transpose(pt, xs[:, d * P:(d + 1) * P], ident)
                      if d % 2 == 0:
                          nc.vector.tensor_copy(
                              out=xts[d][:, st * P:(st + 1) * P], in_=pt)
                      else:
                          nc.scalar.activation(
                              out=xts[d][:, st * P:(st + 1) * P], in_=pt,
                              func=Act.Identity)

              # phase-C weight warm-start (emitted after the x loads so it
              # doesn't delay the startup-critical DMA queue)
              nc.sync.dma_start(out=wo0_pre, in_=wo_he[0:P, :])

              # ---- head loop ----
              for h in range(n_heads):
                  # packed weight tiles: 4 d-tiles per DMA (free dim = (t e))
                  if h == 0:
                      wq_p, wk_p, wv_p = w0["wq"], w0["wk"], w0["wv"]
                  else:
                      wq_p = [wp.tile([P, 4 * DH], F32R, name=f"wq{h}_{g}",
                                      tag=f"wq{g}") for g in range(2)]
                      wk_p = [wp.tile([P, 4 * DH], F32R, name=f"wk{h}_{g}",
                                      tag=f"wk{g}") for g in range(2)]
                      wv_p = [wp.tile([P, 4 * DH], F32R, name=f"wv{h}_{g}",
                                      tag=f"wv{g}") for g in range(2)]
                      for g in range(2):
                          for wpk, wsrc in ((wq_p, wq), (wk_p, wk), (wv_p, wv)):
                              nc.sync.dma_start(
                                  out=wpk[g].rearrange("p (t e) -> p t e", t=4),
                                  in_=wsrc[h, g * 512:(g + 1) * 512, :].rearrange(
                                      "(t p) e -> p t e", p=P))

                  def wslice(wpk, d, lo, hi):
                      return wpk[d // 4][:, (d % 4) * DH + lo:(d % 4) * DH + hi]

                  # per-(e, chunk) tiles: slots release as soon as this
                  # head's scores for that chunk are done, letting the next
                  # head's projections evacuate early
                  qtc = [[qkv.tile([P, QC], F32R, name=f"qt{h}_{e}_{c}",
                                   tag=f"qt{e}c{c}")
                          for c in range(NQC)] for e in range(NT_E)]
                  kt = [qkv.tile([P, S], F32R, name=f"kt{h}_{e}", tag=f"kt{e}")
                        for e in range(NT_E)]
                  vt = [qkv.tile([P, DH], F32R, name=f"v{h}_{s}", tag=f"v{s}")
                        for s in range(NT_S)]

                  # QT / KT projections: [dh, S] (psum on "med", which is
                  # idle during the previous head's attention)
                  for isq, (wts, b_sb) in enumerate(((wq_p, bq_sb),
                                                     (wk_p, bk_sb))):
                      for e in range(NT_E):
                          for qi in range(NQC):
                              pp = ps1.tile([P, QC], F32,
                                            name=f"pp{h}_{isq}_{e}_{qi}",
                                            tag="med", bufs=2)
                              for d in range(NT_D):
                                  nc.tensor.matmul(
                                      pp,
                                      wslice(wts, d, e * P, (e + 1) * P),
                                      xts[d][:, qi * QC:(qi + 1) * QC],
                                      start=(d == 0), stop=(d == NT_D - 1))
                              tgt = (qtc[e][qi] if isq == 0
                                     else kt[e][:, qi * QC:(qi + 1) * QC])
                              nc.scalar.activation(
                                  out=tgt, in_=pp,
                                  func=Act.Identity,
                                  bias=b_sb[:, h * NT_E + e:h * NT_E + e + 1])

                  # V projection: [S, dh]
                  for si in range(NT_S):
                      pv = ps1.tile([P, DH], F32, name=f"pv{h}_{si}", tag="med", bufs=2)
                      for d in range(NT_D):
                          nc.tensor.matmul(
                              pv, xts[d][:, si * P:(si + 1) * P],
                              wslice(wv_p, d, 0, DH),
                              start=(d == 0), stop=(d == NT_D - 1))
                      nc.scalar.activation(out=vt[si], in_=pv, func=Act.Identity)

                  # attention, q-chunked
                  for qi in range(NQC):
                      es = [ep.tile([P, QC], F32R, name=f"e{h}_{qi}_{k}", tag=f"e{k}")
                            for k in range(NT_S)]
                      for k in range(NT_S):
                          psc = ps1.tile([P, QC], F32,
                                         name=f"sc{h}_{qi}_{k}", tag="pj", bufs=3)
                          for e in range(NT_E):
                              nc.tensor.matmul(
                                  psc,
                                  kt[e][:, k * P:(k + 1) * P],
                                  qtc[e][qi],
                                  start=(e == 0), stop=(e == NT_E - 1))
                          nc.scalar.activation(out=es[k], in_=psc,
                                               func=Act.Exp, scale=SCALE)

                      pzs = [ps1.tile([P, QC], F32,
                                      name=f"pz{h}_{qi}_{e}", tag="z", bufs=3)
                             for e in range(NT_E)]
                      for k in range(NT_S):
                          for e in range(NT_E):
                              nc.tensor.matmul(
                                  pzs[e], vt[k][:, e * P:(e + 1) * P], es[k],
                                  start=(k == 0), stop=(k == NT_S - 1))

                      # evacuate unnormalized z from PSUM right away (ACT)
                      # so the PSUM slots don't wait for the denominator
                      zu = [work.tile([P, QC], F32, name=f"zu{h}_{qi}_{e}",
                                      tag=f"zu{e}", bufs=2)
                            for e in range(NT_E)]
                      for e in range(NT_E):
                          nc.scalar.activation(out=zu[e], in_=pzs[e],
                                               func=Act.Identity)
                      # denominator: two parallel DVE accumulate chains
                      # (hidden under the z matmuls) + GpSimd partition reduce
                      da = [work.tile([P, QC], F32, name=f"da{h}_{qi}_{j}",
                                      tag=f"dacc{j}", bufs=1) for j in range(2)]
                      hn = NT_S // 2
                      for j in range(2):
                          nc.vector.tensor_add(da[j],
                                               es[j * hn].bitcast(F32),
                                               es[j * hn + 1].bitcast(F32))
                          for k in range(j * hn + 2, j * hn + hn):
                              nc.vector.tensor_add(da[j], da[j],
                                                   es[k].bitcast(F32))
                      nc.vector.tensor_add(da[0], da[0], da[1])
                      denb = work.tile([P, QC], F32,
                                       name=f"db{h}_{qi}", tag="denb", bufs=1)
                      nc.gpsimd.partition_all_reduce(
                          denb, da[0], channels=P,
                          reduce_op=bass_isa.ReduceOp.add)
                      rb = work.tile([P, QC], F32, name=f"rb{h}_{qi}", tag="rb", bufs=1)
                      nc.vector.reciprocal(out=rb, in_=denb)
                      for e in range(NT_E):
                          znt = work.tile([P, QC], F32R,
                                          name=f"zn{h}_{qi}_{e}", tag="zn", bufs=2)
                          nc.vector.tensor_mul(znt, zu[e], rb)
                          nc.sync.dma_start(
                              out=zn[h * DH + e * P:h * DH + (e + 1) * P,
                                     qi * QC:(qi + 1) * QC],
                              in_=znt)

          # ---- phase C: transposed output projection outT[d, s] ----
          if do_c:
           with (
              tc.tile_pool(name="wop", bufs=1) as wop,
              tc.tile_pool(name="zsp", bufs=1) as zsp,
              tc.tile_pool(name="osp", bufs=4) as osp,
              tc.tile_pool(name="ps2", bufs=1, space="PSUM") as ps2,
            ):
              wo_p = [wop.tile([P, 2 * D], F32R, name=f"wop{g}", tag=f"wop{g}")
                      for g in range(4)]
              nc.sync.dma_start(
                  out=wo_p[0][:, D:2 * D],
                  in_=wo_he[P:2 * P, :])
              for g in range(1, 4):
                  nc.sync.dma_start(
                      out=wo_p[g].rearrange("p (t e) -> p t e", t=2),
                      in_=wo_he[2 * g * P:2 * (g + 1) * P, :].rearrange(
                          "(t p) e -> p t e", p=P))

              def wo_slice(i, lo, hi):
                  if i == 0:
                      return wo0_pre[:, lo:hi]
                  return wo_p[i // 2][:, (i % 2) * D + lo:(i % 2) * D + hi]
              for sc in range(NQC):
                  zsl = [zsp.tile([P, QC], F32R, name=f"zs{sc}_{i}",
                                  tag=f"zs{i}", bufs=2)
                         for i in range(NHE)]
                  for i in range(NHE):
                      nc.sync.dma_start(
                          out=zsl[i],
                          in_=zn[i * P:(i + 1) * P, sc * QC:(sc + 1) * QC])
                  for dt in range(NT_D):
                      po = ps2.tile([P, QC], F32, name=f"po{sc}_{dt}", tag="o", bufs=6)
                      for i in range(NHE):
                          nc.tensor.matmul(
                              po, wo_slice(i, dt * P, (dt + 1) * P), zsl[i],
                              start=(i == 0), stop=(i == NHE - 1))
                      ost = osp.tile([P, QC], F32, name=f"os{sc}_{dt}", tag="ost")
                      nc.scalar.activation(out=ost, in_=po, func=Act.Identity,
                                           bias=cb_sb[:, dt:dt + 1])
                      nc.sync.dma_start(
                          out=outT[dt * P:(dt + 1) * P, sc * QC:(sc + 1) * QC],
                          in_=ost)

    nc.compile()
    return nc


_CACHE = {}


def _get_nc():
    key = (os.environ.get("KBENCH_REPS", "1"),
           os.environ.get("KBUILD_HEADS"), os.environ.get("KBUILD_NTS"),
           os.environ.get("KBUILD_PHASE_C"))
    if _CACHE.get("key") != key:
        _CACHE["nc"] = _build()
        _CACHE["key"] = key
    return _CACHE["nc"]


LAST_RESULTS = None


def kernel(**inputs) -> np.ndarray:
    x = np.ascontiguousarray(np.asarray(inputs["normalized_resid_pre"],
                                        dtype=np.float32))
    n = x.shape[0]
    assert x.shape == (N_CORES, S, D), x.shape
    w_o = np.ascontiguousarray(np.asarray(inputs["W_O"], np.float32))
    b_v = np.asarray(inputs["b_V"], np.float32)
    b_o = np.asarray(inputs["b_O"], np.float32)
    # bias of the output projection, folded with b_V's contribution through W_O
    cb = b_o + np.tensordot(b_v, w_o, axes=([0, 1], [0, 1])).astype(np.float32)
    base = {
        "wq": np.ascontiguousarray(np.asarray(inputs["W_Q"], np.float32)),
        "wk": np.ascontiguousarray(np.asarray(inputs["W_K"], np.float32)),
        "wv": np.ascontiguousarray(np.asarray(inputs["W_V"], np.float32)),
        "wo": w_o,
        "bq": np.ascontiguousarray(np.asarray(inputs["b_Q"], np.float32)),
        "bk": np.ascontiguousarray(np.asarray(inputs["b_K"], np.float32)),
        "cb": np.ascontiguousarray(cb),
        "idin": np.eye(P, dtype=np.float32),
    }
    nc = _get_nc()
    in_maps = [dict(base, x=x[i]) for i in range(n)]
    trace = os.environ.get("KERNEL_TRACE", "0") == "1"
    res = run_bass_kernel_spmd(nc, in_maps, core_ids=list(range(N_CORES)),
                               trace=trace)
    global LAST_RESULTS
    LAST_RESULTS = res
    return np.stack([res.results[i]["outT"].T for i in range(n)], axis=0)

